# revision 16
# baseline (speedup 1.0000x reference)
"""SAGAN-style self-attention block on 8 trn2 NeuronCores.

Full inputs: x [8, 512, 64, 64], w_theta [64, 512], w_phi [64, 512],
w_g [256, 512], w_o [512, 256], gamma scalar.

Sharding: data-parallel over batch - one batch item per core, identical
program, weights replicated.

Per-core math (C=512, n=H*W=4096, m=n/4=1024):
  theta = w_theta @ x            [64, 4096]   (f32r)
  phi   = pool2(w_phi @ x)       [64, 1024]   (f32r)
  g     = pool2(w_g @ x)         [256, 1024]  (bf16)
  S^T   = phi^T @ theta          [1024, 4096] scores, m-major layout
  E     = exp(S^T)               (bf16; |S| < ~50 so no max-subtraction)
  Z     = ones^T @ (tree-sum E)  row sums: 1 matmul per slice
  att   = (g @ E) * (1/Z)        [256, 4096]
  out   = (gamma*w_o) @ att + x  [512, 4096]  (bf16 out, f32 on host)

The cost model charges matmuls by output columns only (fp32r at >=256
free is full rate), so dtype does not change PE time; bf16 is used
where it halves DVE time / SBUF. DMA is one serialized pipe (~360GB/s,
~630ns fixed HWDGE cost per transfer), so x streams in for ~25us.

Schedule: 8 column slices of 512. Scores+exps for slices 0-3 run inside
phase 1 (projections, which is otherwise DMA-gated) and slices 4-7
spread over the phase-2 windows:

  w_k PE:  [Z(k+1)] [sc x attend(k+1) ct0] [outproj(k) ot0,ot1]
           [sc x attend(k+1) ct1] [outproj(k) ot2,ot3]
  w_k DVE: rcp(k+1); att-mul(k+1) per ct right after that ct's attend
           stops (outproj(k) reads att(k) computed a full window
           earlier and never stalls); residual adds; tree-sum(k+2)
  w_k Pool: E pair-sums (SBUF only - gpsimd cannot touch PSUM)
  w_k ACT: exps, theta copies (late slices' theta goes to DVE instead;
           ACT is exp-bound feeding the prologue there)

PSUM: score/Z ring 4 banks + shared attend/outproj ring 4 banks = 8.
Output DMAs are batched 4-row-blocks per slice (one strided descriptor
set) to amortize the fixed HWDGE cost; the final window's op(6)
residuals drain via ACT-copy+Pool-add on the scalar queue so the DVE
can chase op(7)'s residuals at their dependency floor.
"""

import time
from contextlib import ExitStack

import numpy as np

import bass_rust
import concourse.bass as bass
import concourse.mybir as mybir
import concourse.tile as tile
from concourse.bass_utils import run_bass_kernel_spmd
from concourse.masks import make_identity

P = 128
C = 512  # channels
C8 = 64  # theta/phi channels
C2 = 256  # g channels
N = 4096  # H*W
M = 1024  # pooled spatial
NS = 8  # n-slices
SL = 512  # n-slice width
MT = 8  # m-tiles of 128
F32 = mybir.dt.float32
F32R = mybir.dt.float32r
BF16 = mybir.dt.bfloat16
AX = mybir.AxisListType
ALU = mybir.AluOpType
ACTF = mybir.ActivationFunctionType

# phase-1 score pull-in: interleaved at proj-group boundaries of slice ns.
# Constraint: i <= ns-1 and mt <= ns-1 (theta(i) and phi(mt) must exist).
PH1_SCORES = {
    1: [(0, 0)],
    2: [(0, 1), (1, 0), (1, 1)],
    3: [(0, 2), (1, 2), (2, 0), (2, 1), (2, 2)],
    4: [(0, 3), (1, 3), (2, 3), (3, 0), (3, 1)],
    5: [(3, 2), (3, 3), (0, 4), (1, 4), (2, 4)],
    6: [(3, 4), (0, 5), (1, 5), (2, 5), (3, 5)],
    7: [(0, 6), (1, 6), (2, 6), (3, 6)],
}
# scores for slices 4-7 spread over phase-2 windows (deadline: exps of
# slice s done before attend(s) in window s-1; Z-tree of s by window s-1)
WIN_SCORES = {
    0: [(4, 0), (4, 1), (4, 2), (4, 3), (4, 4)],
    1: [(4, 5), (4, 6), (4, 7), (5, 0), (5, 1), (5, 2)],
    2: [(5, 3), (5, 4), (5, 5), (5, 6), (5, 7)],
    3: [(6, 0), (6, 1), (6, 2), (6, 3), (6, 4), (6, 5)],
    4: [(6, 6), (6, 7), (7, 0), (7, 1), (7, 2), (7, 3), (7, 4), (7, 5), (7, 6), (7, 7)],
    5: [],
    6: [],
}


def _pool_view(ap):
    """[p, 512] slice of the conv output -> 5D maxpool view [p, h2, w2, dy, dx].

    Within an n-slice of 512 = 8 image rows: local n = (2*h2+dy)*64 + 2*w2+dx.
    """
    return ap.rearrange("p (h2 dy w2 dx) -> p h2 w2 dy dx", h2=4, dy=2, w2=32, dx=2)


def emit(nc, tc, ctx):
    x_f = nc.dram_tensor("x", [C, N], F32R, kind="ExternalInput")
    wproj = nc.dram_tensor("wproj", [C, 384], F32R, kind="ExternalInput")
    wo = nc.dram_tensor("wo", [C2, C], F32, kind="ExternalInput")
    out_d = nc.dram_tensor("out", [C, N], BF16, kind="ExternalOutput")

    persist = ctx.enter_context(tc.tile_pool(name="persist", bufs=1))

    # ---- input DMA order (single serialized DMA pipe): slice-0 x chunks
    # interleaved with wproj k-chunks (slice-0 projections run k-outer so
    # they start on the first pair), then x slices 1-7, then wo.
    xf = [persist.tile([P, N], F32R, name=f"xf{cc}") for cc in range(4)]
    wpt = persist.tile([P, 4, 384], F32R, name="wpt")
    for cc in range(4):
        nc.sync.dma_start(
            out=wpt[:, cc, :], in_=wproj[cc * P : (cc + 1) * P, :]
        )
        nc.sync.dma_start(out=xf[cc][:, 0:SL], in_=x_f[cc * P : (cc + 1) * P, 0:SL])
    wp = [wpt[:, k, :] for k in range(4)]
    for q in range(1, NS):
        for cc in range(4):
            nc.sync.dma_start(
                out=xf[cc][:, q * SL : (q + 1) * SL],
                in_=x_f[cc * P : (cc + 1) * P, q * SL : (q + 1) * SL],
            )
    wot_f = []
    for k in range(2):
        t = persist.tile([P, C], F32, name=f"wotf{k}")
        nc.sync.dma_start(out=t, in_=wo[k * P : (k + 1) * P, :])
        wot_f.append(t)

    # ---- constants
    ones_f = persist.tile([P, P], F32)
    nc.vector.memset(ones_f, 1.0)
    ones_bf = persist.tile([P, P], BF16)
    nc.vector.memset(ones_bf, 1.0)
    ident_bf = persist.tile([P, P], BF16)
    make_identity(nc, ident_bf)
    wot = []
    for k in range(2):
        t = persist.tile([P, C], BF16, name=f"wot{k}")
        nc.vector.tensor_copy(t, wot_f[k])
        wot.append(t)

    # persistent activations
    theta = persist.tile([C8, N], F32R)
    phi = persist.tile([C8, M], F32R)
    g = [persist.tile([P, M], BF16, name=f"g{i}") for i in range(2)]
    gT = [persist.tile([P, C2], BF16, name=f"gT{mt}") for mt in range(MT)]

    # score/Z psum ring persists across phases
    spool = ctx.enter_context(tc.tile_pool(name="spsum", bufs=1, space="PSUM"))
    etp = ctx.enter_context(tc.tile_pool(name="et", bufs=4))
    miscp = ctx.enter_context(tc.tile_pool(name="misc", bufs=2))

    ET = [[None] * MT for _ in range(NS)]
    FS = [[None] * (MT // 2) for _ in range(NS)]
    TS = [None] * NS
    RINV = [None] * NS
    APT = [[None, None] for _ in range(NS)]
    ATT = [[None, None] for _ in range(NS)]

    # Warm-up: keep the PE p-state ramp going while the first x tiles land.
    # Warm tiles live in the score ring (same shape, no readers).
    for wi in range(5):
        wt_ = spool.tile([P, SL], F32, name="warm", tag="sw", bufs=4)
        nc.tensor.matmul(
            wt_[:, 0:P], lhsT=ones_f, rhs=ones_f, start=True, stop=True,
            skip_group_check=True,
        )

    def emit_score(i, mt, pair_eng=None):
        """One score matmul (K=64) + exp for slice i, m-tile mt; on odd mt
        also the E pair-sum (Pool by default) feeding the Z adder tree."""
        nsl = slice(i * SL, (i + 1) * SL)
        sp = spool.tile([P, SL], F32, name="sp", tag="sw", bufs=4)
        nc.tensor.matmul(
            sp,
            lhsT=phi[:, mt * P : (mt + 1) * P],
            rhs=theta[:, nsl],
            start=True,
            stop=True,
            skip_group_check=True,
        )
        et = etp.tile([P, SL], BF16, name="et", tag=f"et{mt}")
        nc.scalar.activation(et, sp, ACTF.Exp)
        ET[i][mt] = et
        if mt % 2 == 1:
            eng = pair_eng or nc.gpsimd
            f = miscp.tile([P, SL], BF16, name="fs", tag=f"fs{mt // 2}", bufs=3)
            eng.tensor_add(f, ET[i][mt - 1], ET[i][mt])
            FS[i][mt // 2] = f

    def emit_tree(i, eng=None):
        # quad+final sums -> TS[i], the Z matmul's rhs (SBUF-only, so any
        # vector engine is legal)
        eng = eng or nc.vector
        q0 = miscp.tile([P, SL], BF16, name="q0", tag="q0")
        eng.tensor_add(q0, FS[i][0], FS[i][1])
        q1 = miscp.tile([P, SL], BF16, name="q1", tag="q1")
        eng.tensor_add(q1, FS[i][2], FS[i][3])
        t = miscp.tile([P, SL], BF16, name="ts", tag="ts", bufs=3)
        eng.tensor_add(t, q0, q1)
        TS[i] = t

    def emit_z(i):
        # Z row-sums via one K=128 matmul; reciprocal on DVE right after
        zp = spool.tile([P, SL], F32, name="zp", tag="sw", bufs=4)
        nc.tensor.matmul(
            zp, lhsT=ones_bf, rhs=TS[i], start=True, stop=True,
            skip_group_check=True,
        )
        r = miscp.tile([P, SL], F32, name="rinv", tag="rinv")
        nc.vector.reciprocal(r, zp)
        RINV[i] = r

    def emit_attmul(i, ct):
        # normalize: att = ap * (1/Z), right after attend ct stops
        a = miscp.tile([P, SL], BF16, name="att", tag=f"att{ct}")
        nc.vector.tensor_mul(a, APT[i][ct], RINV[i])
        ATT[i][ct] = a

    def emit_attend_ct(i, ct, sc_list, qp):
        """attend matmuls for slice i, one ct half, with score singles
        interleaved; followed by the normalization mul on DVE."""
        APT[i][ct] = qp.tile([P, SL], F32, name="ap", tag="apop", bufs=4)
        for h in range(2):
            for _ in range(2):
                if sc_list:
                    emit_score(*sc_list.pop(0))
            for mt in range(4 * h, 4 * h + 4):
                nc.tensor.matmul(
                    APT[i][ct],
                    lhsT=gT[mt][:, ct * P : (ct + 1) * P],
                    rhs=ET[i][mt],
                    start=(mt == 0),
                    stop=(mt == MT - 1),
                    skip_group_check=True,
                )
        emit_attmul(i, ct)

    OBM = [None] * NS

    def emit_outproj(i, ots, res_eng, qp, merged=False, split=False):
        nsl = slice(i * SL, (i + 1) * SL)
        for ot in ots:
            op_ = qp.tile([P, SL], F32, name="op", tag="apop", bufs=4)
            for ct in range(2):
                nc.tensor.matmul(
                    op_,
                    lhsT=wot[ct][:, ot * P : (ot + 1) * P],
                    rhs=ATT[i][ct],
                    start=(ct == 0),
                    stop=(ct == 1),
                    skip_group_check=True,
                )
            if merged:
                # one strided DMA per slice instead of four (HWDGE costs a
                # fixed ~625ns per dma_start, so batch the four ot blocks)
                if OBM[i] is None:
                    OBM[i] = miscp.tile([P, 4, SL], BF16, name="obm", tag="obm")
                res_eng.tensor_add(
                    OBM[i][:, ot, :], op_, xf[ot][:, nsl].bitcast(F32)
                )
                if ot == 3:
                    nc.sync.dma_start(
                        out=out_d.ap().rearrange("(u p) n -> p u n", u=4)[:, :, nsl],
                        in_=OBM[i],
                    )
            elif split:
                # tail offload: ACT drains the psum, Pool adds the residual,
                # DMA rides the scalar queue (no SP SEQ head-of-line) - keeps
                # DVE free to chase the final slice's residuals
                obs_f = miscp.tile([P, SL], F32, name="obs", tag=f"obs{ot % 2}")
                nc.scalar.copy(out=obs_f, in_=op_)
                ob = miscp.tile([P, SL], BF16, name="ob", tag=f"ob{ot % 2}", bufs=4)
                nc.gpsimd.tensor_add(ob, obs_f, xf[ot][:, nsl].bitcast(F32))
                nc.scalar.dma_start(out=out_d[ot * P : (ot + 1) * P, nsl], in_=ob)
            else:
                ob = miscp.tile([P, SL], BF16, name="ob", tag=f"ob{ot % 2}", bufs=4)
                res_eng.tensor_add(ob, op_, xf[ot][:, nsl].bitcast(F32))
                nc.sync.dma_start(out=out_d[ot * P : (ot + 1) * P, nsl], in_=ob)

    # ---- phase 1: projections + pooling + g transposes + scores(0..3) ----
    with tc.tile_pool(name="ppsum", bufs=1, space="PSUM") as pp, tc.tile_pool(
        name="tpsum", bufs=1, space="PSUM"
    ) as tp:
        for ns in range(NS):
            nsl = slice(ns * SL, (ns + 1) * SL)
            msl = slice(ns * P, (ns + 1) * P)
            xr = [xf[k][:, nsl] for k in range(4)]
            ps = [pp.tile([P, SL], F32, name="pp", tag=f"pp{mt}") for mt in range(3)]
            sc = [(si, mt, None) for (si, mt) in PH1_SCORES.get(ns, ())]
            if ns == 0:
                # k-outer so each (wproj chunk, x chunk) DMA pair unblocks
                # three matmuls immediately
                for k in range(4):
                    for mt in (1, 2, 0):
                        nc.tensor.matmul(
                            ps[mt],
                            lhsT=wp[k][:, mt * P : (mt + 1) * P],
                            rhs=xr[k],
                            start=(k == 0),
                            stop=(k == 3),
                            skip_group_check=True,
                        )
            else:
                for gi, mt in enumerate((1, 2, 0)):
                    for k in range(4):
                        nc.tensor.matmul(
                            ps[mt],
                            lhsT=wp[k][:, mt * P : (mt + 1) * P],
                            rhs=xr[k],
                            start=(k == 0),
                            stop=(k == 3),
                        )
                    take = 2 if gi < 2 else len(sc)
                    for _ in range(min(take, len(sc))):
                        emit_score(*sc.pop(0))
            # g pools first (their psums are ready first; they gate the
            # transposes), then phi, then the theta copy (ACT)
            for i in range(2):
                nc.vector.tensor_reduce(
                    out=g[i][:, msl],
                    in_=_pool_view(ps[1 + i]),
                    axis=AX.XY,
                    op=ALU.max,
                )
            nc.vector.tensor_reduce(
                out=phi[:, msl],
                in_=_pool_view(ps[0][C8:P, :]),
                axis=AX.XY,
                op=ALU.max,
            )
            if ns >= NS - 3:
                # late slices: ACT is exp-bound feeding the prologue and w0;
                # DVE has slack here
                nc.vector.tensor_copy(out=theta[:, nsl], in_=ps[0][0:C8, :])
            else:
                nc.scalar.copy(out=theta[:, nsl], in_=ps[0][0:C8, :])
            t = tp.tile([P, C2], BF16, name="tp", tag="tp")
            for i in range(2):
                nc.tensor.matmul(
                    t[:, i * P : (i + 1) * P],
                    lhsT=g[i][:, msl],
                    rhs=ident_bf,
                    is_transpose=True,
                    skip_group_check=True,
                )
            nc.vector.tensor_copy(out=gT[ns], in_=t)
            for (si, mt, eng) in sc:
                emit_score(si, mt, pair_eng=eng)

    # ---- phase 2: prologue + pipelined windows -------------------------
    with tc.tile_pool(name="qpsum", bufs=1, space="PSUM") as qp:
        # prologue: last score column (needs slice-7 phi), attend(0), Z(0)
        emit_score(3, 7)
        emit_score(0, 7, pair_eng=nc.vector)
        emit_tree(0)
        emit_score(1, 7, pair_eng=nc.vector)
        emit_tree(1)
        emit_score(2, 7)
        APT[0][0] = qp.tile([P, SL], F32, name="ap", tag="apop", bufs=4)
        for mt in range(MT):
            nc.tensor.matmul(
                APT[0][0],
                lhsT=gT[mt][:, 0:P],
                rhs=ET[0][mt],
                start=(mt == 0),
                stop=(mt == MT - 1),
                skip_group_check=True,
            )
        emit_z(0)
        emit_attmul(0, 0)
        APT[0][1] = qp.tile([P, SL], F32, name="ap", tag="apop", bufs=4)
        for mt in range(MT):
            nc.tensor.matmul(
                APT[0][1],
                lhsT=gT[mt][:, P:C2],
                rhs=ET[0][mt],
                start=(mt == 0),
                stop=(mt == MT - 1),
                skip_group_check=True,
            )
        emit_attmul(0, 1)

        # windows k=0..6
        for k in range(NS - 1):
            sc = list(WIN_SCORES.get(k, ()))
            emit_z(k + 1)
            emit_attend_ct(k + 1, 0, sc, qp)
            if k == 5:
                emit_tree(7, eng=nc.gpsimd)  # early: Z(7) sits at w6's head
            if k == 6:
                # tail: outproj(6) first (att(6) has been ready since w5) so
                # its residuals+DMAs drain during the attend(7) block
                emit_outproj(6, (0, 1), nc.vector, qp)
                emit_outproj(6, (2, 3), nc.vector, qp, split=True)
                emit_attend_ct(7, 1, sc, qp)
                break
            emit_outproj(k, (0, 1), nc.vector, qp, merged=True)
            emit_attend_ct(k + 1, 1, sc, qp)
            while sc:
                emit_score(*sc.pop(0))
            emit_outproj(k, (2, 3), nc.vector, qp, merged=True)
            if k + 2 < NS and k != 5:
                emit_tree(k + 2)

        # epilogue: slice 7 output projection (att muls ran inside w6)
        emit_outproj(NS - 1, (0,), nc.vector, qp)
        emit_outproj(NS - 1, (1,), nc.vector, qp)
        emit_outproj(NS - 1, (2,), nc.vector, qp)
        emit_outproj(NS - 1, (3,), nc.vector, qp)


def build_nc():
    nc = bass.Bass(target_bir_lowering=False, trn_type="TRN2")
    with tile.TileContext(nc) as tc:
        with ExitStack() as ctx:
            emit(nc, tc, ctx)
    bass_rust.generate_event_semaphores(nc)
    return nc


def kernel(x, w_theta, w_phi, w_g, w_o, gamma):
    x = np.asarray(x, dtype=np.float32)
    B = x.shape[0]
    wproj = np.ascontiguousarray(
        np.concatenate(
            [np.asarray(w_theta).T, np.asarray(w_phi).T, np.asarray(w_g).T], axis=1
        ),
        dtype=np.float32,
    )
    wo_t = np.ascontiguousarray(
        (np.float32(gamma) * np.asarray(w_o)).T, dtype=np.float32
    )

    nc = build_nc()
    in_maps = []
    for b in range(B):
        xb = np.ascontiguousarray(x[b].reshape(C, N))
        in_maps.append({"x": xb, "wproj": wproj, "wo": wo_t})
    # retry: rare transient NRT_EXEC_UNIT_UNRECOVERABLE from stale device
    # state clears on re-execution
    last_err = None
    for attempt in range(3):
        try:
            res = run_bass_kernel_spmd(nc, in_maps, core_ids=list(range(B)))
            break
        except Exception as e:  # noqa: BLE001
            last_err = e
            time.sleep(2.0)
    else:
        raise last_err
    out = np.stack(
        [np.asarray(res.results[b]["out"]).astype(np.float32).reshape(C, 64, 64)
         for b in range(B)]
    )
    return out


# revision 17
# speedup vs baseline: 1.0009x; 1.0009x over previous
"""SAGAN-style self-attention block on 8 trn2 NeuronCores.

Full inputs: x [8, 512, 64, 64], w_theta [64, 512], w_phi [64, 512],
w_g [256, 512], w_o [512, 256], gamma scalar.

Sharding: data-parallel over batch - one batch item per core, identical
program, weights replicated.

Per-core math (C=512, n=H*W=4096, m=n/4=1024):
  theta = w_theta @ x            [64, 4096]   (f32r)
  phi   = pool2(w_phi @ x)       [64, 1024]   (f32r)
  g     = pool2(w_g @ x)         [256, 1024]  (bf16)
  S^T   = phi^T @ theta          [1024, 4096] scores, m-major layout
  E     = exp(S^T)               (bf16; |S| < ~50 so no max-subtraction)
  Z     = ones^T @ (tree-sum E)  row sums: 1 matmul per slice
  att   = (g @ E) * (1/Z)        [256, 4096]
  out   = (gamma*w_o) @ att + x  [512, 4096]  (bf16 out, f32 on host)

The cost model charges matmuls by output columns only (fp32r at >=256
free is full rate), so dtype does not change PE time; bf16 is used
where it halves DVE time / SBUF. DMA is one serialized pipe (~360GB/s,
~630ns fixed HWDGE cost per transfer), so x streams in for ~25us.

Schedule: 8 column slices of 512. Scores+exps for slices 0-3 run inside
phase 1 (projections, which is otherwise DMA-gated) and slices 4-7
spread over the phase-2 windows:

  w_k PE:  [Z(k+1)] [sc x attend(k+1) ct0] [outproj(k) ot0,ot1]
           [sc x attend(k+1) ct1] [outproj(k) ot2,ot3]
  w_k DVE: rcp(k+1); att-mul(k+1) per ct right after that ct's attend
           stops (outproj(k) reads att(k) computed a full window
           earlier and never stalls); residual adds; tree-sum(k+2)
  w_k Pool: E pair-sums (SBUF only - gpsimd cannot touch PSUM)
  w_k ACT: exps, theta copies (late slices' theta goes to DVE instead;
           ACT is exp-bound feeding the prologue there)

PSUM: score/Z ring 4 banks + shared attend/outproj ring 4 banks = 8.
Output DMAs are batched 4-row-blocks per slice (one strided descriptor
set) to amortize the fixed HWDGE cost; the final window's op(6)
residuals drain via ACT-copy+Pool-add on the scalar queue so the DVE
can chase op(7)'s residuals at their dependency floor.
"""

import time
from contextlib import ExitStack

import numpy as np

import bass_rust
import concourse.bass as bass
import concourse.mybir as mybir
import concourse.tile as tile
from concourse.bass_utils import run_bass_kernel_spmd
from concourse.masks import make_identity

P = 128
C = 512  # channels
C8 = 64  # theta/phi channels
C2 = 256  # g channels
N = 4096  # H*W
M = 1024  # pooled spatial
NS = 8  # n-slices
SL = 512  # n-slice width
MT = 8  # m-tiles of 128
F32 = mybir.dt.float32
F32R = mybir.dt.float32r
BF16 = mybir.dt.bfloat16
AX = mybir.AxisListType
ALU = mybir.AluOpType
ACTF = mybir.ActivationFunctionType

# phase-1 score pull-in: interleaved at proj-group boundaries of slice ns.
# Constraint: i <= ns-1 and mt <= ns-1 (theta(i) and phi(mt) must exist).
PH1_SCORES = {
    1: [(0, 0)],
    2: [(0, 1), (1, 0), (1, 1)],
    3: [(0, 2), (1, 2), (2, 0), (2, 1), (2, 2)],
    4: [(0, 3), (1, 3), (2, 3), (3, 0), (3, 1)],
    5: [(3, 2), (3, 3), (0, 4), (1, 4), (2, 4)],
    6: [(3, 4), (0, 5), (1, 5), (2, 5), (3, 5)],
    7: [(0, 6), (1, 6), (2, 6), (3, 6)],
}
# scores for slices 4-7 spread over phase-2 windows (deadline: exps of
# slice s done before attend(s) in window s-1; Z-tree of s by window s-1)
WIN_SCORES = {
    0: [(4, 0), (4, 1), (4, 2), (4, 3), (4, 4)],
    1: [(4, 5), (4, 6), (4, 7), (5, 0), (5, 1), (5, 2)],
    2: [(5, 3), (5, 4), (5, 5), (5, 6), (5, 7)],
    3: [(6, 0), (6, 1), (6, 2), (6, 3), (6, 4), (6, 5)],
    4: [(6, 6), (6, 7), (7, 0), (7, 1), (7, 2), (7, 3), (7, 4), (7, 5), (7, 6), (7, 7)],
    5: [],
    6: [],
}


def _pool_view(ap):
    """[p, 512] slice of the conv output -> 5D maxpool view [p, h2, w2, dy, dx].

    Within an n-slice of 512 = 8 image rows: local n = (2*h2+dy)*64 + 2*w2+dx.
    """
    return ap.rearrange("p (h2 dy w2 dx) -> p h2 w2 dy dx", h2=4, dy=2, w2=32, dx=2)


def emit(nc, tc, ctx):
    x_f = nc.dram_tensor("x", [C, N], F32R, kind="ExternalInput")
    wproj = nc.dram_tensor("wproj", [C, 384], F32R, kind="ExternalInput")
    wo = nc.dram_tensor("wo", [C2, C], F32, kind="ExternalInput")
    out_d = nc.dram_tensor("out", [C, N], BF16, kind="ExternalOutput")

    persist = ctx.enter_context(tc.tile_pool(name="persist", bufs=1))

    # ---- input DMA order (single serialized DMA pipe): slice-0 x chunks
    # interleaved with wproj k-chunks (slice-0 projections run k-outer so
    # they start on the first pair), then x slices 1-7, then wo.
    xf = [persist.tile([P, N], F32R, name=f"xf{cc}") for cc in range(4)]
    wpt = persist.tile([P, 4, 384], F32R, name="wpt")
    for cc in range(4):
        eng = nc.gpsimd if cc == 0 else nc.sync
        eng.dma_start(
            out=wpt[:, cc, :], in_=wproj[cc * P : (cc + 1) * P, :]
        )
        eng.dma_start(out=xf[cc][:, 0:SL], in_=x_f[cc * P : (cc + 1) * P, 0:SL])
    wp = [wpt[:, k, :] for k in range(4)]
    for q in range(1, NS):
        for cc in range(4):
            nc.sync.dma_start(
                out=xf[cc][:, q * SL : (q + 1) * SL],
                in_=x_f[cc * P : (cc + 1) * P, q * SL : (q + 1) * SL],
            )
    wot_f = []
    for k in range(2):
        t = persist.tile([P, C], F32, name=f"wotf{k}")
        nc.sync.dma_start(out=t, in_=wo[k * P : (k + 1) * P, :])
        wot_f.append(t)

    # ---- constants
    ones_f = persist.tile([P, P], F32)
    nc.vector.memset(ones_f, 1.0)
    ones_bf = persist.tile([P, P], BF16)
    nc.vector.memset(ones_bf, 1.0)
    ident_bf = persist.tile([P, P], BF16)
    make_identity(nc, ident_bf)
    wot = []
    for k in range(2):
        t = persist.tile([P, C], BF16, name=f"wot{k}")
        nc.vector.tensor_copy(t, wot_f[k])
        wot.append(t)

    # persistent activations
    theta = persist.tile([C8, N], F32R)
    phi = persist.tile([C8, M], F32R)
    g = [persist.tile([P, M], BF16, name=f"g{i}") for i in range(2)]
    gT = [persist.tile([P, C2], BF16, name=f"gT{mt}") for mt in range(MT)]

    # score/Z psum ring persists across phases
    spool = ctx.enter_context(tc.tile_pool(name="spsum", bufs=1, space="PSUM"))
    etp = ctx.enter_context(tc.tile_pool(name="et", bufs=4))
    miscp = ctx.enter_context(tc.tile_pool(name="misc", bufs=2))

    ET = [[None] * MT for _ in range(NS)]
    FS = [[None] * (MT // 2) for _ in range(NS)]
    TS = [None] * NS
    RINV = [None] * NS
    APT = [[None, None] for _ in range(NS)]
    ATT = [[None, None] for _ in range(NS)]

    # Warm-up: keep the PE p-state ramp going while the first x tiles land.
    # Warm tiles live in the score ring (same shape, no readers).
    for wi in range(5):
        wt_ = spool.tile([P, SL], F32, name="warm", tag="sw", bufs=4)
        nc.tensor.matmul(
            wt_[:, 0:P], lhsT=ones_f, rhs=ones_f, start=True, stop=True,
            skip_group_check=True,
        )

    def emit_score(i, mt, pair_eng=None):
        """One score matmul (K=64) + exp for slice i, m-tile mt; on odd mt
        also the E pair-sum (Pool by default) feeding the Z adder tree."""
        nsl = slice(i * SL, (i + 1) * SL)
        sp = spool.tile([P, SL], F32, name="sp", tag="sw", bufs=4)
        nc.tensor.matmul(
            sp,
            lhsT=phi[:, mt * P : (mt + 1) * P],
            rhs=theta[:, nsl],
            start=True,
            stop=True,
            skip_group_check=True,
        )
        et = etp.tile([P, SL], BF16, name="et", tag=f"et{mt}")
        nc.scalar.activation(et, sp, ACTF.Exp)
        ET[i][mt] = et
        if mt % 2 == 1:
            eng = pair_eng or nc.gpsimd
            f = miscp.tile([P, SL], BF16, name="fs", tag=f"fs{mt // 2}", bufs=3)
            eng.tensor_add(f, ET[i][mt - 1], ET[i][mt])
            FS[i][mt // 2] = f

    def emit_tree(i, eng=None):
        # quad+final sums -> TS[i], the Z matmul's rhs (SBUF-only, so any
        # vector engine is legal)
        eng = eng or nc.vector
        q0 = miscp.tile([P, SL], BF16, name="q0", tag="q0")
        eng.tensor_add(q0, FS[i][0], FS[i][1])
        q1 = miscp.tile([P, SL], BF16, name="q1", tag="q1")
        eng.tensor_add(q1, FS[i][2], FS[i][3])
        t = miscp.tile([P, SL], BF16, name="ts", tag="ts", bufs=3)
        eng.tensor_add(t, q0, q1)
        TS[i] = t

    def emit_z(i):
        # Z row-sums via one K=128 matmul; reciprocal on DVE right after
        zp = spool.tile([P, SL], F32, name="zp", tag="sw", bufs=4)
        nc.tensor.matmul(
            zp, lhsT=ones_bf, rhs=TS[i], start=True, stop=True,
            skip_group_check=True,
        )
        r = miscp.tile([P, SL], F32, name="rinv", tag="rinv")
        nc.vector.reciprocal(r, zp)
        RINV[i] = r

    def emit_attmul(i, ct):
        # normalize: att = ap * (1/Z), right after attend ct stops
        a = miscp.tile([P, SL], BF16, name="att", tag=f"att{ct}")
        nc.vector.tensor_mul(a, APT[i][ct], RINV[i])
        ATT[i][ct] = a

    def emit_attend_ct(i, ct, sc_list, qp):
        """attend matmuls for slice i, one ct half, with score singles
        interleaved; followed by the normalization mul on DVE."""
        APT[i][ct] = qp.tile([P, SL], F32, name="ap", tag="apop", bufs=4)
        for h in range(2):
            for _ in range(2):
                if sc_list:
                    emit_score(*sc_list.pop(0))
            for mt in range(4 * h, 4 * h + 4):
                nc.tensor.matmul(
                    APT[i][ct],
                    lhsT=gT[mt][:, ct * P : (ct + 1) * P],
                    rhs=ET[i][mt],
                    start=(mt == 0),
                    stop=(mt == MT - 1),
                    skip_group_check=True,
                )
        emit_attmul(i, ct)

    OBM = [None] * NS

    def emit_outproj(i, ots, res_eng, qp, merged=False, split=False):
        nsl = slice(i * SL, (i + 1) * SL)
        for ot in ots:
            op_ = qp.tile([P, SL], F32, name="op", tag="apop", bufs=4)
            for ct in range(2):
                nc.tensor.matmul(
                    op_,
                    lhsT=wot[ct][:, ot * P : (ot + 1) * P],
                    rhs=ATT[i][ct],
                    start=(ct == 0),
                    stop=(ct == 1),
                    skip_group_check=True,
                )
            if merged:
                # one strided DMA per slice instead of four (HWDGE costs a
                # fixed ~625ns per dma_start, so batch the four ot blocks)
                if OBM[i] is None:
                    OBM[i] = miscp.tile([P, 4, SL], BF16, name="obm", tag="obm")
                res_eng.tensor_add(
                    OBM[i][:, ot, :], op_, xf[ot][:, nsl].bitcast(F32)
                )
                if ot == 3:
                    nc.sync.dma_start(
                        out=out_d.ap().rearrange("(u p) n -> p u n", u=4)[:, :, nsl],
                        in_=OBM[i],
                    )
            elif split:
                # tail offload: ACT drains the psum, Pool adds the residual,
                # DMA rides the scalar queue (no SP SEQ head-of-line) - keeps
                # DVE free to chase the final slice's residuals
                obs_f = miscp.tile([P, SL], F32, name="obs", tag=f"obs{ot % 2}")
                nc.scalar.copy(out=obs_f, in_=op_)
                ob = miscp.tile([P, SL], BF16, name="ob", tag=f"ob{ot % 2}", bufs=4)
                nc.gpsimd.tensor_add(ob, obs_f, xf[ot][:, nsl].bitcast(F32))
                nc.scalar.dma_start(out=out_d[ot * P : (ot + 1) * P, nsl], in_=ob)
            else:
                ob = miscp.tile([P, SL], BF16, name="ob", tag=f"ob{ot % 2}", bufs=4)
                res_eng.tensor_add(ob, op_, xf[ot][:, nsl].bitcast(F32))
                nc.sync.dma_start(out=out_d[ot * P : (ot + 1) * P, nsl], in_=ob)

    # ---- phase 1: projections + pooling + g transposes + scores(0..3) ----
    with tc.tile_pool(name="ppsum", bufs=1, space="PSUM") as pp, tc.tile_pool(
        name="tpsum", bufs=1, space="PSUM"
    ) as tp:
        for ns in range(NS):
            nsl = slice(ns * SL, (ns + 1) * SL)
            msl = slice(ns * P, (ns + 1) * P)
            xr = [xf[k][:, nsl] for k in range(4)]
            ps = [pp.tile([P, SL], F32, name="pp", tag=f"pp{mt}") for mt in range(3)]
            sc = [(si, mt, None) for (si, mt) in PH1_SCORES.get(ns, ())]
            if ns == 0:
                # k-outer so each (wproj chunk, x chunk) DMA pair unblocks
                # three matmuls immediately
                for k in range(4):
                    for mt in (1, 2, 0):
                        nc.tensor.matmul(
                            ps[mt],
                            lhsT=wp[k][:, mt * P : (mt + 1) * P],
                            rhs=xr[k],
                            start=(k == 0),
                            stop=(k == 3),
                            skip_group_check=True,
                        )
            else:
                for gi, mt in enumerate((1, 2, 0)):
                    for k in range(4):
                        nc.tensor.matmul(
                            ps[mt],
                            lhsT=wp[k][:, mt * P : (mt + 1) * P],
                            rhs=xr[k],
                            start=(k == 0),
                            stop=(k == 3),
                        )
                    take = 2 if gi < 2 else len(sc)
                    for _ in range(min(take, len(sc))):
                        emit_score(*sc.pop(0))
            # g pools first (their psums are ready first; they gate the
            # transposes), then phi, then the theta copy (ACT)
            for i in range(2):
                nc.vector.tensor_reduce(
                    out=g[i][:, msl],
                    in_=_pool_view(ps[1 + i]),
                    axis=AX.XY,
                    op=ALU.max,
                )
            nc.vector.tensor_reduce(
                out=phi[:, msl],
                in_=_pool_view(ps[0][C8:P, :]),
                axis=AX.XY,
                op=ALU.max,
            )
            if ns >= NS - 3:
                # late slices: ACT is exp-bound feeding the prologue and w0;
                # DVE has slack here
                nc.vector.tensor_copy(out=theta[:, nsl], in_=ps[0][0:C8, :])
            else:
                nc.scalar.copy(out=theta[:, nsl], in_=ps[0][0:C8, :])
            t = tp.tile([P, C2], BF16, name="tp", tag="tp")
            for i in range(2):
                nc.tensor.matmul(
                    t[:, i * P : (i + 1) * P],
                    lhsT=g[i][:, msl],
                    rhs=ident_bf,
                    is_transpose=True,
                    skip_group_check=True,
                )
            nc.vector.tensor_copy(out=gT[ns], in_=t)
            for (si, mt, eng) in sc:
                emit_score(si, mt, pair_eng=eng)

    # ---- phase 2: prologue + pipelined windows -------------------------
    with tc.tile_pool(name="qpsum", bufs=1, space="PSUM") as qp:
        # prologue: last score column (needs slice-7 phi), attend(0), Z(0)
        emit_score(3, 7)
        emit_score(0, 7, pair_eng=nc.vector)
        emit_tree(0)
        emit_score(1, 7, pair_eng=nc.vector)
        emit_tree(1)
        emit_score(2, 7)
        APT[0][0] = qp.tile([P, SL], F32, name="ap", tag="apop", bufs=4)
        for mt in range(MT):
            nc.tensor.matmul(
                APT[0][0],
                lhsT=gT[mt][:, 0:P],
                rhs=ET[0][mt],
                start=(mt == 0),
                stop=(mt == MT - 1),
                skip_group_check=True,
            )
        emit_z(0)
        emit_attmul(0, 0)
        APT[0][1] = qp.tile([P, SL], F32, name="ap", tag="apop", bufs=4)
        for mt in range(MT):
            nc.tensor.matmul(
                APT[0][1],
                lhsT=gT[mt][:, P:C2],
                rhs=ET[0][mt],
                start=(mt == 0),
                stop=(mt == MT - 1),
                skip_group_check=True,
            )
        emit_attmul(0, 1)

        # windows k=0..6
        for k in range(NS - 1):
            sc = list(WIN_SCORES.get(k, ()))
            emit_z(k + 1)
            emit_attend_ct(k + 1, 0, sc, qp)
            if k == 5:
                emit_tree(7, eng=nc.gpsimd)  # early: Z(7) sits at w6's head
            if k == 6:
                # tail: outproj(6) first (att(6) has been ready since w5) so
                # its residuals+DMAs drain during the attend(7) block
                emit_outproj(6, (0, 1), nc.vector, qp)
                emit_outproj(6, (2, 3), nc.vector, qp, split=True)
                emit_attend_ct(7, 1, sc, qp)
                break
            emit_outproj(k, (0, 1), nc.vector, qp, merged=True)
            emit_attend_ct(k + 1, 1, sc, qp)
            while sc:
                emit_score(*sc.pop(0))
            emit_outproj(k, (2, 3), nc.vector, qp, merged=True)
            if k + 2 < NS and k != 5:
                emit_tree(k + 2)

        # epilogue: slice 7 output projection (att muls ran inside w6)
        emit_outproj(NS - 1, (0,), nc.vector, qp)
        emit_outproj(NS - 1, (1,), nc.vector, qp)
        emit_outproj(NS - 1, (2,), nc.vector, qp)
        emit_outproj(NS - 1, (3,), nc.vector, qp)


def build_nc():
    nc = bass.Bass(target_bir_lowering=False, trn_type="TRN2")
    with tile.TileContext(nc) as tc:
        with ExitStack() as ctx:
            emit(nc, tc, ctx)
    bass_rust.generate_event_semaphores(nc)
    return nc


def kernel(x, w_theta, w_phi, w_g, w_o, gamma):
    x = np.asarray(x, dtype=np.float32)
    B = x.shape[0]
    wproj = np.ascontiguousarray(
        np.concatenate(
            [np.asarray(w_theta).T, np.asarray(w_phi).T, np.asarray(w_g).T], axis=1
        ),
        dtype=np.float32,
    )
    wo_t = np.ascontiguousarray(
        (np.float32(gamma) * np.asarray(w_o)).T, dtype=np.float32
    )

    nc = build_nc()
    in_maps = []
    for b in range(B):
        xb = np.ascontiguousarray(x[b].reshape(C, N))
        in_maps.append({"x": xb, "wproj": wproj, "wo": wo_t})
    # retry: rare transient NRT_EXEC_UNIT_UNRECOVERABLE from stale device
    # state clears on re-execution
    last_err = None
    for attempt in range(3):
        try:
            res = run_bass_kernel_spmd(nc, in_maps, core_ids=list(range(B)))
            break
        except Exception as e:  # noqa: BLE001
            last_err = e
            time.sleep(2.0)
    else:
        raise last_err
    out = np.stack(
        [np.asarray(res.results[b]["out"]).astype(np.float32).reshape(C, 64, 64)
         for b in range(B)]
    )
    return out


# revision 18
# speedup vs baseline: 1.0055x; 1.0047x over previous
"""SAGAN-style self-attention block on 8 trn2 NeuronCores.

Full inputs: x [8, 512, 64, 64], w_theta [64, 512], w_phi [64, 512],
w_g [256, 512], w_o [512, 256], gamma scalar.

Sharding: data-parallel over batch - one batch item per core, identical
program, weights replicated.

Per-core math (C=512, n=H*W=4096, m=n/4=1024):
  theta = w_theta @ x            [64, 4096]   (f32r)
  phi   = pool2(w_phi @ x)       [64, 1024]   (f32r)
  g     = pool2(w_g @ x)         [256, 1024]  (bf16)
  S^T   = phi^T @ theta          [1024, 4096] scores, m-major layout
  E     = exp(S^T)               (bf16; |S| < ~50 so no max-subtraction)
  Z     = ones^T @ (tree-sum E)  row sums: 1 matmul per slice
  att   = (g @ E) * (1/Z)        [256, 4096]
  out   = (gamma*w_o) @ att + x  [512, 4096]  (bf16 out, f32 on host)

The cost model charges matmuls by output columns only (fp32r at >=256
free is full rate), so dtype does not change PE time; bf16 is used
where it halves DVE time / SBUF. DMA is one serialized pipe (~360GB/s,
~630ns fixed HWDGE cost per transfer), so x streams in for ~25us.

Schedule: 8 column slices of 512. Scores+exps for slices 0-3 run inside
phase 1 (projections, which is otherwise DMA-gated) and slices 4-7
spread over the phase-2 windows:

  w_k PE:  [Z(k+1)] [sc x attend(k+1) ct0] [outproj(k) ot0,ot1]
           [sc x attend(k+1) ct1] [outproj(k) ot2,ot3]
  w_k DVE: rcp(k+1); att-mul(k+1) per ct right after that ct's attend
           stops (outproj(k) reads att(k) computed a full window
           earlier and never stalls); residual adds; tree-sum(k+2)
  w_k Pool: E pair-sums (SBUF only - gpsimd cannot touch PSUM)
  w_k ACT: exps, theta copies (late slices' theta goes to DVE instead;
           ACT is exp-bound feeding the prologue there)

PSUM: score/Z ring 4 banks + shared attend/outproj ring 4 banks = 8.
Output DMAs are batched 4-row-blocks per slice (one strided descriptor
set) to amortize the fixed HWDGE cost; the final window's op(6)
residuals drain via ACT-copy+Pool-add on the scalar queue so the DVE
can chase op(7)'s residuals at their dependency floor.
"""

import time
from contextlib import ExitStack

import numpy as np

import bass_rust
import concourse.bass as bass
import concourse.mybir as mybir
import concourse.tile as tile
from concourse.bass_utils import run_bass_kernel_spmd
from concourse.masks import make_identity

P = 128
C = 512  # channels
C8 = 64  # theta/phi channels
C2 = 256  # g channels
N = 4096  # H*W
M = 1024  # pooled spatial
NS = 8  # n-slices
SL = 512  # n-slice width
MT = 8  # m-tiles of 128
F32 = mybir.dt.float32
F32R = mybir.dt.float32r
BF16 = mybir.dt.bfloat16
AX = mybir.AxisListType
ALU = mybir.AluOpType
ACTF = mybir.ActivationFunctionType

# phase-1 score pull-in: interleaved at proj-group boundaries of slice ns.
# Constraint: i <= ns-1 and mt <= ns-1 (theta(i) and phi(mt) must exist).
PH1_SCORES = {
    1: [(0, 0)],
    2: [(0, 1), (1, 0), (1, 1)],
    3: [(0, 2), (1, 2), (2, 0), (2, 1), (2, 2)],
    4: [(0, 3), (1, 3), (2, 3), (3, 0), (3, 1)],
    5: [(3, 2), (3, 3), (0, 4), (1, 4), (2, 4)],
    6: [(3, 4), (0, 5), (1, 5), (2, 5), (3, 5)],
    7: [(0, 6), (1, 6), (2, 6), (3, 6)],
}
# scores for slices 4-7 spread over phase-2 windows (deadline: exps of
# slice s done before attend(s) in window s-1; Z-tree of s by window s-1)
WIN_SCORES = {
    0: [(4, 0), (4, 1), (4, 2), (4, 3), (4, 4)],
    1: [(4, 5), (4, 6), (4, 7), (5, 0), (5, 1), (5, 2)],
    2: [(5, 3), (5, 4), (5, 5), (5, 6), (5, 7)],
    3: [(6, 0), (6, 1), (6, 2), (6, 3), (6, 4), (6, 5)],
    4: [(6, 6), (6, 7), (7, 0), (7, 1), (7, 2), (7, 3), (7, 4), (7, 5), (7, 6), (7, 7)],
    5: [],
    6: [],
}


def _pool_view(ap):
    """[p, 512] slice of the conv output -> 5D maxpool view [p, h2, w2, dy, dx].

    Within an n-slice of 512 = 8 image rows: local n = (2*h2+dy)*64 + 2*w2+dx.
    """
    return ap.rearrange("p (h2 dy w2 dx) -> p h2 w2 dy dx", h2=4, dy=2, w2=32, dx=2)


def emit(nc, tc, ctx):
    x_f = nc.dram_tensor("x", [C, N], F32R, kind="ExternalInput")
    wproj = nc.dram_tensor("wproj", [C, 384], F32R, kind="ExternalInput")
    wo = nc.dram_tensor("wo", [C2, C], F32, kind="ExternalInput")
    out_d = nc.dram_tensor("out", [C, N], BF16, kind="ExternalOutput")

    persist = ctx.enter_context(tc.tile_pool(name="persist", bufs=1))

    # ---- input DMA order (single serialized DMA pipe): slice-0 x chunks
    # interleaved with wproj k-chunks (slice-0 projections run k-outer so
    # they start on the first pair), then x slices 1-7, then wo.
    xf = [persist.tile([P, N], F32R, name=f"xf{cc}") for cc in range(4)]
    wpt = persist.tile([P, 4, 384], F32R, name="wpt")
    for cc in range(4):
        eng = nc.gpsimd if cc == 0 else nc.sync
        eng.dma_start(
            out=wpt[:, cc, :], in_=wproj[cc * P : (cc + 1) * P, :]
        )
        eng.dma_start(out=xf[cc][:, 0:SL], in_=x_f[cc * P : (cc + 1) * P, 0:SL])
    wp = [wpt[:, k, :] for k in range(4)]
    for q in range(1, NS):
        for cc in range(4):
            nc.sync.dma_start(
                out=xf[cc][:, q * SL : (q + 1) * SL],
                in_=x_f[cc * P : (cc + 1) * P, q * SL : (q + 1) * SL],
            )
    wot_f = []
    for k in range(2):
        t = persist.tile([P, C], F32, name=f"wotf{k}")
        nc.sync.dma_start(out=t, in_=wo[k * P : (k + 1) * P, :])
        wot_f.append(t)

    # ---- constants
    ones_f = persist.tile([P, P], F32)
    nc.vector.memset(ones_f, 1.0)
    ones_bf = persist.tile([P, P], BF16)
    nc.vector.memset(ones_bf, 1.0)
    ident_bf = persist.tile([P, P], BF16)
    make_identity(nc, ident_bf)
    wot = []
    for k in range(2):
        t = persist.tile([P, C], BF16, name=f"wot{k}")
        nc.vector.tensor_copy(t, wot_f[k])
        wot.append(t)

    # persistent activations
    theta = persist.tile([C8, N], F32R)
    phi = persist.tile([C8, M], F32R)
    g = [persist.tile([P, M], BF16, name=f"g{i}") for i in range(2)]
    gT = [persist.tile([P, C2], BF16, name=f"gT{mt}") for mt in range(MT)]

    # score/Z psum ring persists across phases
    spool = ctx.enter_context(tc.tile_pool(name="spsum", bufs=1, space="PSUM"))
    etp = ctx.enter_context(tc.tile_pool(name="et", bufs=4))
    miscp = ctx.enter_context(tc.tile_pool(name="misc", bufs=2))

    ET = [[None] * MT for _ in range(NS)]
    FS = [[None] * (MT // 2) for _ in range(NS)]
    TS = [None] * NS
    RINV = [None] * NS
    APT = [[None, None] for _ in range(NS)]
    ATT = [[None, None] for _ in range(NS)]

    # Warm-up: keep the PE p-state ramp going while the first x tiles land.
    # Warm tiles live in the score ring (same shape, no readers).
    for wi in range(5):
        wt_ = spool.tile([P, SL], F32, name="warm", tag="sw", bufs=4)
        nc.tensor.matmul(
            wt_[:, 0:P], lhsT=ones_f, rhs=ones_f, start=True, stop=True,
            skip_group_check=True,
        )

    def emit_score(i, mt, pair_eng=None):
        """One score matmul (K=64) + exp for slice i, m-tile mt; on odd mt
        also the E pair-sum (Pool by default) feeding the Z adder tree."""
        nsl = slice(i * SL, (i + 1) * SL)
        sp = spool.tile([P, SL], F32, name="sp", tag="sw", bufs=4)
        nc.tensor.matmul(
            sp,
            lhsT=phi[:, mt * P : (mt + 1) * P],
            rhs=theta[:, nsl],
            start=True,
            stop=True,
            skip_group_check=True,
        )
        et = etp.tile([P, SL], BF16, name="et", tag=f"et{mt}")
        nc.scalar.activation(et, sp, ACTF.Exp)
        ET[i][mt] = et
        if mt % 2 == 1:
            eng = pair_eng or nc.gpsimd
            f = miscp.tile([P, SL], BF16, name="fs", tag=f"fs{mt // 2}", bufs=3)
            eng.tensor_add(f, ET[i][mt - 1], ET[i][mt])
            FS[i][mt // 2] = f

    def emit_tree(i, eng=None):
        # quad+final sums -> TS[i], the Z matmul's rhs (SBUF-only, so any
        # vector engine is legal)
        eng = eng or nc.vector
        q0 = miscp.tile([P, SL], BF16, name="q0", tag="q0")
        eng.tensor_add(q0, FS[i][0], FS[i][1])
        q1 = miscp.tile([P, SL], BF16, name="q1", tag="q1")
        eng.tensor_add(q1, FS[i][2], FS[i][3])
        t = miscp.tile([P, SL], BF16, name="ts", tag="ts", bufs=3)
        eng.tensor_add(t, q0, q1)
        TS[i] = t

    def emit_z(i):
        # Z row-sums via one K=128 matmul; reciprocal on DVE right after
        zp = spool.tile([P, SL], F32, name="zp", tag="sw", bufs=4)
        nc.tensor.matmul(
            zp, lhsT=ones_bf, rhs=TS[i], start=True, stop=True,
            skip_group_check=True,
        )
        r = miscp.tile([P, SL], F32, name="rinv", tag="rinv")
        nc.vector.reciprocal(r, zp)
        RINV[i] = r

    def emit_attmul(i, ct):
        # normalize: att = ap * (1/Z), right after attend ct stops
        a = miscp.tile([P, SL], BF16, name="att", tag=f"att{ct}")
        nc.vector.tensor_mul(a, APT[i][ct], RINV[i])
        ATT[i][ct] = a

    def emit_attend_ct(i, ct, sc_list, qp):
        """attend matmuls for slice i, one ct half, with score singles
        interleaved; followed by the normalization mul on DVE."""
        APT[i][ct] = qp.tile([P, SL], F32, name="ap", tag="apop", bufs=4)
        for h in range(2):
            for _ in range(2):
                if sc_list:
                    emit_score(*sc_list.pop(0))
            for mt in range(4 * h, 4 * h + 4):
                nc.tensor.matmul(
                    APT[i][ct],
                    lhsT=gT[mt][:, ct * P : (ct + 1) * P],
                    rhs=ET[i][mt],
                    start=(mt == 0),
                    stop=(mt == MT - 1),
                    skip_group_check=True,
                )
        emit_attmul(i, ct)

    OBM = [None] * NS

    def emit_outproj(i, ots, res_eng, qp, merged=False, split=False):
        nsl = slice(i * SL, (i + 1) * SL)
        for ot in ots:
            op_ = qp.tile([P, SL], F32, name="op", tag="apop", bufs=4)
            for ct in range(2):
                nc.tensor.matmul(
                    op_,
                    lhsT=wot[ct][:, ot * P : (ot + 1) * P],
                    rhs=ATT[i][ct],
                    start=(ct == 0),
                    stop=(ct == 1),
                    skip_group_check=True,
                )
            if merged:
                # one strided DMA per slice instead of four (HWDGE costs a
                # fixed ~625ns per dma_start, so batch the four ot blocks)
                if OBM[i] is None:
                    OBM[i] = miscp.tile([P, 4, SL], BF16, name="obm", tag="obm")
                res_eng.tensor_add(
                    OBM[i][:, ot, :], op_, xf[ot][:, nsl].bitcast(F32)
                )
                if ot == 3:
                    nc.sync.dma_start(
                        out=out_d.ap().rearrange("(u p) n -> p u n", u=4)[:, :, nsl],
                        in_=OBM[i],
                    )
            elif split:
                # tail offload: ACT drains the psum, Pool adds the residual,
                # DMA rides the scalar queue (no SP SEQ head-of-line) - keeps
                # DVE free to chase the final slice's residuals
                obs_f = miscp.tile([P, SL], F32, name="obs", tag=f"obs{ot % 2}")
                nc.scalar.copy(out=obs_f, in_=op_)
                ob = miscp.tile([P, SL], BF16, name="ob", tag=f"ob{ot % 2}", bufs=4)
                nc.gpsimd.tensor_add(ob, obs_f, xf[ot][:, nsl].bitcast(F32))
                nc.scalar.dma_start(out=out_d[ot * P : (ot + 1) * P, nsl], in_=ob)
            else:
                ob = miscp.tile([P, SL], BF16, name="ob", tag=f"ob{ot % 2}", bufs=4)
                res_eng.tensor_add(ob, op_, xf[ot][:, nsl].bitcast(F32))
                nc.sync.dma_start(out=out_d[ot * P : (ot + 1) * P, nsl], in_=ob)

    # ---- phase 1: projections + pooling + g transposes + scores(0..3) ----
    with tc.tile_pool(name="ppsum", bufs=1, space="PSUM") as pp, tc.tile_pool(
        name="tpsum", bufs=1, space="PSUM"
    ) as tp:
        for ns in range(NS):
            nsl = slice(ns * SL, (ns + 1) * SL)
            msl = slice(ns * P, (ns + 1) * P)
            xr = [xf[k][:, nsl] for k in range(4)]
            ps = [pp.tile([P, SL], F32, name="pp", tag=f"pp{mt}") for mt in range(3)]
            sc = [(si, mt, None) for (si, mt) in PH1_SCORES.get(ns, ())]
            if ns == 0:
                # k-outer so each (wproj chunk, x chunk) DMA pair unblocks
                # three matmuls immediately
                for k in range(4):
                    for mt in (1, 2, 0):
                        nc.tensor.matmul(
                            ps[mt],
                            lhsT=wp[k][:, mt * P : (mt + 1) * P],
                            rhs=xr[k],
                            start=(k == 0),
                            stop=(k == 3),
                            skip_group_check=True,
                        )
            else:
                for gi, mt in enumerate((1, 2, 0)):
                    for k in range(4):
                        nc.tensor.matmul(
                            ps[mt],
                            lhsT=wp[k][:, mt * P : (mt + 1) * P],
                            rhs=xr[k],
                            start=(k == 0),
                            stop=(k == 3),
                        )
                    take = 1 if gi == 0 else (2 if gi == 1 else len(sc))
                    for _ in range(min(take, len(sc))):
                        emit_score(*sc.pop(0))
            # g pools first (their psums are ready first; they gate the
            # transposes), then phi, then the theta copy (ACT)
            for i in range(2):
                nc.vector.tensor_reduce(
                    out=g[i][:, msl],
                    in_=_pool_view(ps[1 + i]),
                    axis=AX.XY,
                    op=ALU.max,
                )
            nc.vector.tensor_reduce(
                out=phi[:, msl],
                in_=_pool_view(ps[0][C8:P, :]),
                axis=AX.XY,
                op=ALU.max,
            )
            if ns >= NS - 3:
                # late slices: ACT is exp-bound feeding the prologue and w0;
                # DVE has slack here
                nc.vector.tensor_copy(out=theta[:, nsl], in_=ps[0][0:C8, :])
            else:
                nc.scalar.copy(out=theta[:, nsl], in_=ps[0][0:C8, :])
            t = tp.tile([P, C2], BF16, name="tp", tag="tp")
            for i in range(2):
                nc.tensor.matmul(
                    t[:, i * P : (i + 1) * P],
                    lhsT=g[i][:, msl],
                    rhs=ident_bf,
                    is_transpose=True,
                    skip_group_check=True,
                )
            nc.vector.tensor_copy(out=gT[ns], in_=t)
            for (si, mt, eng) in sc:
                emit_score(si, mt, pair_eng=eng)

    # ---- phase 2: prologue + pipelined windows -------------------------
    with tc.tile_pool(name="qpsum", bufs=1, space="PSUM") as qp:
        # prologue: last score column (needs slice-7 phi), attend(0), Z(0)
        emit_score(3, 7)
        emit_score(0, 7, pair_eng=nc.vector)
        emit_tree(0)
        emit_score(1, 7, pair_eng=nc.vector)
        emit_tree(1)
        emit_score(2, 7)
        APT[0][0] = qp.tile([P, SL], F32, name="ap", tag="apop", bufs=4)
        for mt in range(MT):
            nc.tensor.matmul(
                APT[0][0],
                lhsT=gT[mt][:, 0:P],
                rhs=ET[0][mt],
                start=(mt == 0),
                stop=(mt == MT - 1),
                skip_group_check=True,
            )
        emit_z(0)
        emit_attmul(0, 0)
        APT[0][1] = qp.tile([P, SL], F32, name="ap", tag="apop", bufs=4)
        for mt in range(MT):
            nc.tensor.matmul(
                APT[0][1],
                lhsT=gT[mt][:, P:C2],
                rhs=ET[0][mt],
                start=(mt == 0),
                stop=(mt == MT - 1),
                skip_group_check=True,
            )
        emit_attmul(0, 1)

        # windows k=0..6
        for k in range(NS - 1):
            sc = list(WIN_SCORES.get(k, ()))
            emit_z(k + 1)
            emit_attend_ct(k + 1, 0, sc, qp)
            if k == 5:
                emit_tree(7, eng=nc.gpsimd)  # early: Z(7) sits at w6's head
            if k == 6:
                # tail: outproj(6) first (att(6) has been ready since w5) so
                # its residuals+DMAs drain during the attend(7) block
                emit_outproj(6, (0, 1), nc.vector, qp)
                emit_outproj(6, (2, 3), nc.vector, qp, split=True)
                emit_attend_ct(7, 1, sc, qp)
                break
            emit_outproj(k, (0, 1), nc.vector, qp, merged=True)
            emit_attend_ct(k + 1, 1, sc, qp)
            while sc:
                emit_score(*sc.pop(0))
            emit_outproj(k, (2, 3), nc.vector, qp, merged=True)
            if k + 2 < NS and k != 5:
                emit_tree(k + 2)

        # epilogue: slice 7 output projection (att muls ran inside w6)
        emit_outproj(NS - 1, (0,), nc.vector, qp)
        emit_outproj(NS - 1, (1,), nc.vector, qp)
        emit_outproj(NS - 1, (2,), nc.vector, qp)
        emit_outproj(NS - 1, (3,), nc.vector, qp)


def build_nc():
    nc = bass.Bass(target_bir_lowering=False, trn_type="TRN2")
    with tile.TileContext(nc) as tc:
        with ExitStack() as ctx:
            emit(nc, tc, ctx)
    bass_rust.generate_event_semaphores(nc)
    return nc


def kernel(x, w_theta, w_phi, w_g, w_o, gamma):
    x = np.asarray(x, dtype=np.float32)
    B = x.shape[0]
    wproj = np.ascontiguousarray(
        np.concatenate(
            [np.asarray(w_theta).T, np.asarray(w_phi).T, np.asarray(w_g).T], axis=1
        ),
        dtype=np.float32,
    )
    wo_t = np.ascontiguousarray(
        (np.float32(gamma) * np.asarray(w_o)).T, dtype=np.float32
    )

    nc = build_nc()
    in_maps = []
    for b in range(B):
        xb = np.ascontiguousarray(x[b].reshape(C, N))
        in_maps.append({"x": xb, "wproj": wproj, "wo": wo_t})
    # retry: rare transient NRT_EXEC_UNIT_UNRECOVERABLE from stale device
    # state clears on re-execution
    last_err = None
    for attempt in range(3):
        try:
            res = run_bass_kernel_spmd(nc, in_maps, core_ids=list(range(B)))
            break
        except Exception as e:  # noqa: BLE001
            last_err = e
            time.sleep(2.0)
    else:
        raise last_err
    out = np.stack(
        [np.asarray(res.results[b]["out"]).astype(np.float32).reshape(C, 64, 64)
         for b in range(B)]
    )
    return out
